# revision 29
# baseline (speedup 1.0000x reference)
"""Bahdanau attention kernel for Trainium2 (8 NeuronCores, data-parallel over batch).

Reference computation (per batch row b):
    pq      = query @ Wq.T                       # (B, AD)
    hidden  = tanh(pq[:, None, :] + processed_memory)   # (B, T, AD)
    e       = einsum('btd,d->bt', hidden, v)     # (B, T)
    e       = where(mask, -1e30, e)
    out     = softmax(e, axis=1)

Key observation: ~50% of positions have mask=True, and for those the reference
output is *exactly* 0.0 (exp(-1e30 - max) underflows).  So the host gathers
only the unmasked positions per batch (n_b <= ~2130 for this input family),
pads to a static per-group length, and the device only processes the
compacted stream - halving HBM traffic, tanh work and matmul work.  The host
scatters results back and fills masked positions with exact zeros.

Padding is self-masking: pad columns of pm are set to -30*sign(v_d), so
tanh(pq + pad) saturates to -sign(v_d) and the pad energy is exactly
-sum|v_d| ~= -12.8, giving exp(e_pad)/rowsum ~ 1e-9 - negligible in the
denominator; the host scatter discards pad outputs anyway.  No mask tensor
ever reaches the device.

Per-core batches are sorted by unmasked count: the 4 largest form group 0
(padded to P0), the 4 smallest group 1 (padded to P1 <= P0), trimming tanh /
matmul / DMA work on the second group.

Device strategy (per core, 8 batch slots):
  * pm is host-gathered/transposed to [slot, d-block, 128, P0] fp16 so AD
    sits on SBUF partitions; the "+pq" add folds into the ScalarE tanh as a
    per-partition bias; the v-weighted d-reduction is TensorE matmuls with a
    [128,1] fp16 stationary v column, col-tiled so the 4 slots of a group
    land on PSUM partitions 0/32/64/96 of shared single-bank chunk tiles.
  * Chunk tiles evacuate via full-width [128, 512] copies (VectorE, plus
    ScalarE for the tail group where ScalarE is otherwise idle), then one
    partition-strided SBUF->SBUF DMA relays each group into a [128, PF]
    softmax tile (16 rows per slot).
  * Per-group softmax: ScalarE exp with fused accum_out row sums, selector
    matmuls for the cross-partition sum + broadcast, one scale, one store -
    group 0's output is in HBM while group 1 still computes.
"""

import sys

if "/opt/trn_rl_repo" not in sys.path:
    sys.path.insert(0, "/opt/trn_rl_repo")

import numpy as np

import concourse.bacc as bacc
import concourse.bass as bass
import concourse.tile as tile
from concourse import mybir
from concourse.bass_utils import run_bass_kernel_spmd

B, T, QD, AD = 64, 4096, 1024, 256
NCORES = 8
BLOC = B // NCORES  # batch slots per core
KB = QD // 128      # k-blocks for the pq matmul
DB = AD // 128      # d-blocks (partition blocks of AD)
F32 = mybir.dt.float32
F16 = mybir.dt.float16

NCH = 5  # psum chunk tiles of 512 cols (supports P up to 2560)


def build_nc(P0: int, P1: int) -> bass.Bass:
    assert P0 % 16 == 0 and P1 % 16 == 0 and P1 <= P0 <= NCH * 512
    PF = [P0 // 16, P1 // 16]
    chunk_l = [
        [(c, min(c + 512, Pg)) for c in range(0, Pg, 512)] for Pg in (P0, P1)
    ]

    nc = bacc.Bacc(None, target_bir_lowering=False)

    pm_c = nc.declare_dram_parameter("pm_c", [BLOC, DB, 128, P0], F16, isOutput=False)
    qT = nc.declare_dram_parameter("qT", [128, KB * BLOC], F16, isOutput=False)
    # wq packed per d-block, partition-major: wq_p[db][p, kb*128 + d] =
    # Wq[db*128 + d, kb*128 + p] -> contiguous 2KB DMA rows
    wq_p = [
        nc.declare_dram_parameter(f"wq{db}", [128, KB * 128], F16, isOutput=False)
        for db in range(DB)
    ]
    v_r = nc.declare_dram_parameter("v_r", [128, DB], F16, isOutput=False)
    sel16_d = nc.declare_dram_parameter("sel16", [128, BLOC], F16, isOutput=False)
    selb_d = nc.declare_dram_parameter("selb", [4, 64], F16, isOutput=False)
    out = nc.declare_dram_parameter("out", [BLOC, P0], F32, isOutput=True)

    Tanh = mybir.ActivationFunctionType.Tanh
    Exp = mybir.ActivationFunctionType.Exp

    with tile.TileContext(nc) as tc:
        with (
            tc.tile_pool(name="singles", bufs=1) as singles,
            tc.tile_pool(name="pm", bufs=6) as pm_pool,
            tc.tile_pool(name="hid", bufs=6) as hid_pool,
            tc.tile_pool(name="es", bufs=2) as es_pool,
            tc.tile_pool(name="ps", bufs=1, space="PSUM") as ps_pool,
        ):
            # dummy tanh first: pulls the ACT_TABLE_LOAD off the critical path
            dummy = singles.tile([128, 1], F32)
            nc.gpsimd.memset(dummy, 0.0)
            dummy2 = singles.tile([128, 1], F32)
            nc.scalar.activation(out=dummy2, in_=dummy, func=Tanh)

            # rows not yet written by exp accum_out must be 0.0 (not garbage)
            # when the per-group row-sum matmul reads the full column
            colsum = singles.tile([128, 1], F32)
            nc.gpsimd.memset(colsum, 0.0)

            # ---- critical-path DMAs split across both queues:
            # sync: wq d-block 0, then the pm stream; gpsimd: wq d-block 1,
            # qT and the small constants ----
            qt_sb = singles.tile([128, KB, BLOC], F16)
            nc.sync.dma_start(
                out=qt_sb, in_=qT[:, :].rearrange("p (kb b) -> p kb b", b=BLOC)
            )
            wq_sb = [singles.tile([128, KB, 128], F16, name=f"wq_sb{db}")
                     for db in range(DB)]
            nc.sync.dma_start(
                out=wq_sb[0],
                in_=wq_p[0][:, :].rearrange("p (kb d) -> p kb d", d=128),
            )
            pm_tiles = {}
            pm_tiles[(0, 0)] = pm_pool.tile([128, P0], F16, name="pm_0_0")
            hp = P0 // 2
            nc.sync.dma_start(
                out=pm_tiles[(0, 0)][:, 0:hp], in_=pm_c[0, 0, :, 0:hp]
            )
            # wq d-block 1 rides the gpsimd queue in parallel; it only gates
            # the second tanh of batch 0
            nc.gpsimd.dma_start(
                out=wq_sb[1],
                in_=wq_p[1][:, :].rearrange("p (kb d) -> p kb d", d=128),
            )
            # second half of pm(0,0) plus pm(0,1)/pm(1,0) ride the scalar
            # HWDGE queue (ScalarE is idle during the head) so the sync
            # queue drains pm(0,0) half 1 - the first tanh's gate - sooner
            nc.scalar.dma_start(
                out=pm_tiles[(0, 0)][:, hp:P0], in_=pm_c[0, 0, :, hp:P0]
            )
            for b in range(2):
                for db in range(DB):
                    if (b, db) == (0, 0):
                        continue
                    pm_sb = pm_pool.tile([128, P0], F16, name=f"pm_{b}_{db}")
                    eng = nc.scalar if (b, db) in ((0, 1), (1, 0)) else nc.sync
                    eng.dma_start(out=pm_sb, in_=pm_c[b, db, :, :])
                    pm_tiles[(b, db)] = pm_sb

            v16 = singles.tile([128, DB], F16)
            nc.gpsimd.dma_start(out=v16, in_=v_r[:, :])
            sel16 = singles.tile([128, BLOC], F16)
            nc.gpsimd.dma_start(out=sel16, in_=sel16_d[:, :])
            selb = singles.tile([4, 64], F16)
            nc.gpsimd.dma_start(out=selb, in_=selb_d[:, :])

            # ---- pq = Wq @ query.T, laid out [d % 128, dblk, b] ----
            pq_sb = singles.tile([128, DB, BLOC], F32)
            for db in range(DB):
                ppq = ps_pool.tile([128, BLOC], F32, tag="pq", bufs=1)
                for k in range(KB):
                    nc.tensor.matmul(
                        ppq,
                        lhsT=wq_sb[db][:, k, :],
                        rhs=qt_sb[:, k, :],
                        start=(k == 0),
                        stop=(k == KB - 1),
                    )
                nc.vector.tensor_copy(out=pq_sb[:, db, :], in_=ppq)

            e2 = singles.tile([128, PF[0]], F32)
            work2 = singles.tile([128, PF[0]], F32)
            rinv = singles.tile([4, 1], F16)
            colsum16 = singles.tile([128, 1], F16)

            # ---- main loop ----
            eps = es = None
            for b in range(BLOC):
                g, j = b // 4, b % 4
                Pg = (P0, P1)[g]
                chunks = chunk_l[g]
                if j == 0:
                    # one single-bank PSUM tile per 512-chunk: separate tiles
                    # keep the DVE evacuation copies from creating false WAR
                    # serialization against later matmuls
                    eps = [
                        ps_pool.tile(
                            [128, c1 - c0], F32, tag=f"c{ci}", bufs=1,
                            name=f"ep{g}_{ci}",
                        )
                        for ci, (c0, c1) in enumerate(chunks)
                    ]
                    es = es_pool.tile([128, Pg], F32, tag="es", name=f"es{g}")
                hids = []
                for db in range(DB):
                    if b < 2:
                        pm_sb = pm_tiles[(b, db)]
                    else:
                        pm_sb = pm_pool.tile([128, Pg], F16, tag="")
                        nc.sync.dma_start(out=pm_sb, in_=pm_c[b, db, :, 0:Pg])
                    h = hid_pool.tile([128, Pg], F16, tag="")
                    if b == 0:
                        hp = P0 // 2
                        for c0, c1 in ((0, hp), (hp, P0)):
                            nc.scalar.activation(
                                out=h[:, c0:c1],
                                in_=pm_sb[:, c0:c1],
                                func=Tanh,
                                bias=pq_sb[:, db, b : b + 1],
                                scale=1.0,
                            )
                    else:
                        nc.scalar.activation(
                            out=h,
                            in_=pm_sb[:, 0:Pg],
                            func=Tanh,
                            bias=pq_sb[:, db, b : b + 1],
                            scale=1.0,
                        )
                    hids.append(h)
                for ci, (c0, c1) in enumerate(chunks):
                    nc.tensor.matmul(
                        eps[ci][32 * j : 32 * j + 1, 0 : c1 - c0],
                        lhsT=v16[:, 0:1],
                        rhs=hids[0][:, c0:c1],
                        start=True,
                        stop=False,
                        tile_position=(0, 32 * j),
                    )
                for ci, (c0, c1) in enumerate(chunks):
                    nc.tensor.matmul(
                        eps[ci][32 * j : 32 * j + 1, 0 : c1 - c0],
                        lhsT=v16[:, 1:2],
                        rhs=hids[1][:, c0:c1],
                        start=False,
                        stop=True,
                        tile_position=(0, 32 * j),
                    )
                    if j == 3:
                        # evacuate each chunk as soon as its last matmul
                        # lands; for the tail group ScalarE is idle, so
                        # alternate engines to halve the copy stream
                        cp = eps[ci][:, 0 : c1 - c0]
                        if g == 1 and ci % 2 == 1:
                            nc.scalar.copy(es[:, c0:c1], cp)
                        else:
                            nc.vector.tensor_copy(out=es[:, c0:c1], in_=cp)
                if j == 3:
                    # relayout all 4 strips into the [128, PF] softmax tile
                    # with one partition-strided DMA
                    nc.sync.dma_start(
                        out=e2[g * 64 : g * 64 + 64, 0 : PF[g]],
                        in_=es[0:97:32, 0:Pg],
                    )
            # per-group softmax chains (deps gate execution; emission order
            # only sets engine FIFO position, after all tanhs)
            for g in range(2):
                lo = g * 64
                nc.scalar.activation(
                    out=work2[lo : lo + 64, 0 : PF[g]],
                    in_=e2[lo : lo + 64, 0 : PF[g]],
                    func=Exp,
                    accum_out=colsum[lo : lo + 64, :],
                )
                with nc.allow_low_precision(reason="fp16 rowsum: 5e-4 err vs 2e-2 budget"):
                    nc.vector.tensor_copy(out=colsum16, in_=colsum)
                psum_rs = ps_pool.tile([4, 1], F32, tag="red", bufs=1)
                nc.tensor.matmul(
                    psum_rs,
                    lhsT=sel16[:, 4 * g : 4 * g + 4],
                    rhs=colsum16,
                    start=True,
                    stop=True,
                )
                with nc.allow_low_precision(reason="fp16 rowsum: 5e-4 err vs 2e-2 budget"):
                    nc.vector.reciprocal(out=rinv, in_=psum_rs)
                psum_ri = ps_pool.tile([128, 1], F32, tag="red", bufs=1)
                nc.tensor.matmul(
                    psum_ri[lo : lo + 64, :],
                    lhsT=selb,
                    rhs=rinv,
                    start=True,
                    stop=True,
                    tile_position=(0, 64 * g),
                )
                nc.vector.tensor_scalar_mul(
                    out=e2[lo : lo + 64, 0 : PF[g]],
                    in0=work2[lo : lo + 64, 0 : PF[g]],
                    scalar1=psum_ri[lo : lo + 64, :],
                )
                nc.sync.dma_start(
                    out=out[4 * g : 4 * g + 4, 0 : (P0, P1)[g]].rearrange(
                        "b (q f) -> b q f", f=PF[g]
                    ),
                    in_=e2[lo : lo + 64, 0 : PF[g]],
                )

    nc.finalize()
    return nc


_CACHE: dict = {}


def _get_nc(P0: int, P1: int) -> bass.Bass:
    if (P0, P1) not in _CACHE:
        _CACHE[(P0, P1)] = build_nc(P0, P1)
    return _CACHE[(P0, P1)]


def _r16(n):
    return max(16, -(-n // 16) * 16)


def prep(query, processed_memory, mask, Wq, v):
    """Host-side shard + compact + sort.  Returns (P0, P1, in_maps, scatter)."""
    query = np.asarray(query, dtype=np.float32)
    pm = np.asarray(processed_memory, dtype=np.float32)
    mask_b = np.asarray(mask).astype(bool)
    Wq = np.asarray(Wq, dtype=np.float32)
    v = np.asarray(v, dtype=np.float32)

    idxs = [np.nonzero(~mask_b[b])[0] for b in range(B)]
    # per-core slot order: batches sorted by unmasked count, largest first;
    # slots 0-3 (group 0) pad to P0, slots 4-7 (group 1) to P1
    orders = []
    for i in range(NCORES):
        ns = [len(idxs[i * BLOC + b]) for b in range(BLOC)]
        orders.append(sorted(range(BLOC), key=lambda b: -ns[b]))
    P0 = _r16(max(len(idxs[i * BLOC + orders[i][0]]) for i in range(NCORES)))
    P1 = _r16(max(len(idxs[i * BLOC + orders[i][4]]) for i in range(NCORES)))
    P0 = max(P0, P1)

    wq_pack = []
    for db in range(DB):
        blk = Wq[db * 128 : (db + 1) * 128, :].T.astype(np.float16)  # (QD, 128)
        wq_pack.append(
            np.ascontiguousarray(
                blk.reshape(KB, 128, 128).transpose(1, 0, 2).reshape(128, KB * 128)
            )
        )
    v_r = np.ascontiguousarray(v.reshape(DB, 128).T.astype(np.float16))
    # self-masking pad column: tanh(pq - 30*sign(v_d)) == -sign(v_d)
    pad_col = (-30.0 * np.sign(v).astype(np.float16)).reshape(DB, 128)
    sel16 = np.zeros((128, BLOC), dtype=np.float16)
    for b in range(BLOC):
        sel16[b * 16 : (b + 1) * 16, b] = 1.0
    selb = np.zeros((4, 64), dtype=np.float16)
    for i in range(4):
        selb[i, i * 16 : (i + 1) * 16] = 1.0

    in_maps = []
    for i in range(NCORES):
        pmc = np.empty((BLOC, DB, 128, P0), dtype=np.float16)
        pmc[:] = pad_col[None, :, :, None]
        q_sl = np.empty((BLOC, QD), dtype=np.float32)
        for s in range(BLOC):
            bg = i * BLOC + orders[i][s]
            ix = idxs[bg]
            n = len(ix)
            pmc[s, :, :, :n] = pm[bg, ix, :].astype(np.float16).T.reshape(DB, 128, n)
            q_sl[s] = query[bg]
        qT16 = np.ascontiguousarray(
            q_sl.T.reshape(KB, 128, BLOC).transpose(1, 0, 2).reshape(128, KB * BLOC)
        ).astype(np.float16)
        in_maps.append(
            {
                "pm_c": pmc,
                "qT": qT16,
                "wq0": wq_pack[0],
                "wq1": wq_pack[1],
                "v_r": v_r,
                "sel16": sel16,
                "selb": selb,
            }
        )
    return P0, P1, in_maps, (idxs, orders)


def run_spmd(P0, P1, in_maps, **kwargs):
    return run_bass_kernel_spmd(
        _get_nc(P0, P1), in_maps, list(range(NCORES)), **kwargs
    )


def scatter_out(res, scatter) -> np.ndarray:
    idxs, orders = scatter
    full = np.zeros((B, T), dtype=np.float32)
    for i in range(NCORES):
        o = res.results[i]["out"]
        for s in range(BLOC):
            bg = i * BLOC + orders[i][s]
            ix = idxs[bg]
            full[bg, ix] = o[s, : len(ix)]
    return full


def kernel(query, processed_memory, mask, Wq, v) -> np.ndarray:
    P0, P1, in_maps, scatter = prep(query, processed_memory, mask, Wq, v)
    res = run_spmd(P0, P1, in_maps)
    return scatter_out(res, scatter)
